# revision 9
# baseline (speedup 1.0000x reference)
"""BioJepa dense transformer on 8 TRN2 NeuronCores — feature-major rewrite.

Data-parallel over batch (B=8 -> 1 element/core). Residual x kept
feature-major in SBUF as xT [128, 6, 3072] f32; projections are
weight-stationary with 512-col token streams; LN stats via ones-matmul +
DVE 32x32 block-transpose Newton rsqrt; action attention reduced to
x += colsum(M) (alpha ~= 1, single action token); optional fp8 e4m3
DoubleRow for projections+MLP.
"""
import numpy as np

import concourse.bass as bass
import concourse.bacc as bacc
import concourse.mybir as mybir
import concourse.tile as tile
from concourse.alu_op_type import AluOpType
from concourse.bass_utils import run_bass_kernel_spmd
from concourse.masks import make_identity

F32 = mybir.dt.float32
BF16 = mybir.dt.bfloat16
F8 = mybir.dt.float8e4
I32 = mybir.dt.int32
AF = mybir.ActivationFunctionType
OP = AluOpType
DR = mybir.MatmulPerfMode.DoubleRow

P = 128
D = 768
KD = 6            # D / 128
T = 3072
NT = 24           # T / 128
H = 12
NPR = 6           # head pairs
FC = 24           # F / 128
L = 6
TT = 1024
CL = 2048
A_PAD = 384
HB = T // 2       # half-token span

MODE = 'bf16'     # 'bf16' | 'mlp8' | 'full8'
REPEAT = 1


def kvoff(pr):
    """kvm psum/sbuf col offset for pair pr (130 cols, 3 pairs per bank)."""
    return (pr // 3) * 512 + (pr % 3) * 130


def build_nc(mode=MODE, repeat=REPEAT, n_layers=L, dump=-1):
    a8 = mode == 'full8'           # attention in fp8
    m8 = mode in ('mlp8', 'full8')  # MLP in fp8
    ADT = F8 if a8 else BF16
    MDT = F8 if m8 else BF16
    SKVM = 1.0 / 256 if a8 else 1.0
    SKBD = 1.0 / 4096 if a8 else 1.0
    SPAN = 1536 if a8 else 1024    # pass2 token span (SBUF-fit)
    NSP = T // SPAN
    NG = SPAN // 512
    SPANM = 1536 if m8 else 1024   # MLP token span
    NSPM = T // SPANM
    NGM = SPANM // 512

    nc = bacc.Bacc()

    x0_d = nc.declare_dram_parameter("x0", [T, D], F32, isOutput=False)
    act_d = nc.declare_dram_parameter("act", [A_PAD, 1], F32, isOutput=False)
    adw1_d = nc.declare_dram_parameter("adw1", [A_PAD, D], F32, isOutput=False)
    adw2_d = nc.declare_dram_parameter("adw2", [D, D], F32, isOutput=False)
    av_d = nc.declare_dram_parameter("av", [L, D, D], BF16, isOutput=False)
    ac_d = nc.declare_dram_parameter("ac", [L, D, D], BF16, isOutput=False)
    sq_d = nc.declare_dram_parameter("sq", [L, D, D], ADT, isOutput=False)
    sk_d = nc.declare_dram_parameter("sk", [L, D, D], ADT, isOutput=False)
    sv_d = nc.declare_dram_parameter("sv", [L, D, D], ADT, isOutput=False)
    sc_d = nc.declare_dram_parameter("sc", [L, D, D], ADT, isOutput=False)
    w1_d = nc.declare_dram_parameter("w1", [L, D, 4 * D], MDT, isOutput=False)
    w2_d = nc.declare_dram_parameter("w2", [L, 4 * D, D], MDT, isOutput=False)
    wmu_d = nc.declare_dram_parameter("wmu", [D, D], BF16, isOutput=False)
    wlv_d = nc.declare_dram_parameter("wlv", [D, D], BF16, isOutput=False)
    mu_d = nc.declare_dram_parameter("mu", [TT, D], F32, isOutput=True)
    lv_d = nc.declare_dram_parameter("lv", [TT, D], F32, isOutput=True)
    dbg_d = (nc.declare_dram_parameter("dbg", [P, KD, T], F32, isOutput=True)
             if dump >= 0 else None)

    def kchunks(nk, f8):
        if f8:
            return [(2 * j, 2) for j in range(nk // 2)]
        return [(c, 1) for c in range(nk)]

    def s3(ap, c0, n, cols):
        """slice [P, K, X] -> [P, cols] (n=1) or [P, n, cols] (n=2)."""
        if n == 1:
            return ap[:, c0, cols]
        return ap[:, c0:c0 + n, cols]

    def cmm(out, lhsT, rhs, start, stop, n, skip=False):
        nc.tensor.matmul(out, lhsT, rhs, start=start, stop=stop,
                         perf_mode=(DR if n == 2 else None),
                         skip_group_check=skip)

    with tile.TileContext(nc) as tc:
        with tc.tile_pool(name="const", bufs=1) as const_p, \
             tc.tile_pool(name="xres", bufs=1) as xres_p, \
             tc.tile_pool(name="lnp", bufs=1) as ln_p, \
             tc.tile_pool(name="attn", bufs=1) as attn_p, \
             tc.tile_pool(name="ps_big", bufs=2, space="PSUM") as big_p:

            ident32 = const_p.tile([P, P], F32, name="ident32")
            make_identity(nc, ident32)
            onesb = const_p.tile([P, 1], BF16, name="onesb")
            nc.vector.memset(onesb, 1.0)
            invd = const_p.tile([P, 1], BF16, name="invd")
            nc.vector.memset(invd, 1.0 / D)
            embT = const_p.tile([P, KD], F32, name="embT")
            # sel16 [2, 128]: row0 = 1 on cols 0:64, row1 = 1 on cols 64:128
            sel16 = const_p.tile([2, P], BF16, name="sel16")

            xT = xres_p.tile([P, KD, T], F32, name="xT")
            lnx16 = ln_p.tile([P, KD, T], BF16, name="lnx16")
            lnxa = lnx16.bitcast(F8)[:, :, 0:T] if a8 else lnx16
            lnxm = lnx16.bitcast(F8)[:, :, 0:T] if m8 else lnx16

            kvm8 = attn_p.tile([P, 1024], ADT, name="kvm8")
            kbd = attn_p.tile([P, H], ADT, name="kbd")
            M_sb = attn_p.tile([12, D], BF16, name="M_sb")
            msumT = attn_p.tile([P, KD], F32, name="msumT")

            def ps():
                return big_p.tile([P, 1536], F32, tag="big", name="ps_big")

            def tr128(ps_out, in_ap):
                pp = in_ap.shape[0]
                b = in_ap.base_partition()
                nc.tensor.transpose(ps_out, in_ap, ident32[b:b + pp, b:b + pp])

            # build sel16 via column memsets + PE transpose
            with tc.tile_pool(name="selb", bufs=1) as selb_p, \
                 tc.tile_pool(name="ps_sel", bufs=1, space="PSUM") as sel_pp:
                colsel = selb_p.tile([P, 2], BF16, name="colsel")
                identb = selb_p.tile([P, P], BF16, name="identb")
                make_identity(nc, identb)
                nc.vector.memset(colsel, 0.0)
                nc.vector.memset(colsel[0:64, 0:1], 1.0)
                nc.vector.memset(colsel[64:P, 1:2], 1.0)
                sel_ps = sel_pp.tile([P, P], BF16, name="sel_ps")
                nc.tensor.transpose(sel_ps[0:2, 0:P], colsel, identb)
                nc.vector.tensor_copy(out=sel16, in_=sel_ps[0:2, 0:P])

            # ---------------- adapter: action -> embT [P, KD] f32 ----------
            with tc.tile_pool(name="wad", bufs=1) as wad_p:
                act_sb = wad_p.tile([P, 3], F32, name="act_sb")
                nc.sync.dma_start(out=act_sb,
                                  in_=act_d.rearrange("(k p) o -> p (k o)", p=P))
                a1_ps = ps()
                for k in range(3):
                    kp = P if k < 2 else 64
                    wt = wad_p.tile([P, D], F32, tag="adw1t", bufs=2)
                    nc.sync.dma_start(out=wt[:kp, :], in_=adw1_d[k * P:k * P + kp, :])
                    nc.tensor.matmul(a1_ps[0:1, 0:512], act_sb[:kp, k:k + 1],
                                     wt[:kp, 0:512], start=k == 0, stop=k == 2)
                    nc.tensor.matmul(a1_ps[0:1, 512:D], act_sb[:kp, k:k + 1],
                                     wt[:kp, 512:D], start=k == 0, stop=k == 2)
                a1 = wad_p.tile([1, D], F32, name="a1")
                nc.vector.tensor_copy(out=a1, in_=a1_ps[0:1, 0:D])
                # LN on the row (bn_stats) + newton rsqrt
                stats = wad_p.tile([1, 3, 6], F32, name="adstats")
                av_ = a1.rearrange("p (s c) -> p s c", s=3)
                for s in range(3):
                    nc.vector.bn_stats(out=stats[0:1, s, :], in_=av_[:, s, :])
                mv1 = wad_p.tile([1, 2], F32, name="mv1")
                nc.vector.bn_aggr(out=mv1, in_=stats[0:1])
                vp = wad_p.tile([1, 4], F32, name="ad_nt")
                nc.vector.tensor_scalar(out=vp[0:1, 0:1], in0=mv1[0:1, 1:2],
                                        scalar1=1e-5, scalar2=None, op0=OP.add)
                y_ = vp[0:1, 1:2]
                yi = y_.bitcast(I32)
                vi = vp[0:1, 0:1].bitcast(I32)
                nc.vector.tensor_scalar(out=yi, in0=vi, scalar1=1,
                                        scalar2=None, op0=OP.arith_shift_right)
                nc.vector.tensor_scalar(out=yi, in0=yi, scalar1=-1,
                                        scalar2=0x5f3759df, op0=OP.mult, op1=OP.add)
                nc.vector.tensor_scalar(out=vp[0:1, 2:3], in0=vp[0:1, 0:1],
                                        scalar1=0.5, scalar2=None, op0=OP.mult)
                for _ in range(3):
                    nc.vector.tensor_tensor(out=vp[0:1, 3:4], in0=y_, in1=y_,
                                            op=OP.mult)
                    nc.vector.tensor_tensor(out=vp[0:1, 3:4], in0=vp[0:1, 3:4],
                                            in1=vp[0:1, 2:3], op=OP.mult)
                    nc.vector.tensor_scalar(out=vp[0:1, 3:4], in0=vp[0:1, 3:4],
                                            scalar1=-1.0, scalar2=1.5,
                                            op0=OP.mult, op1=OP.add)
                    nc.vector.tensor_tensor(out=y_, in0=y_, in1=vp[0:1, 3:4],
                                            op=OP.mult)
                nc.vector.tensor_scalar(out=a1, in0=a1, scalar1=mv1[0:1, 0:1],
                                        scalar2=y_, op0=OP.subtract, op1=OP.mult)
                gl = wad_p.tile([1, D], F32, name="gl")
                nc.scalar.activation(out=gl, in_=a1, func=AF.Gelu, bias=0.0,
                                     scale=1.0)
                # transpose row -> a1T [P, KD] (3 transposes per psum slot)
                a1T = wad_p.tile([P, KD], F32, name="a1T")
                for half in range(2):
                    tp = ps()
                    for j in range(3):
                        k = half * 3 + j
                        tr128(tp[:, j * 512:j * 512 + 1],
                              gl[0:1, k * P:(k + 1) * P])
                    nc.vector.tensor_copy(
                        out=a1T[:, half * 3:half * 3 + 3],
                        in_=tp.rearrange("p (a b) -> p a b", b=512)[:, :, 0])
                # embT = (gl @ adw2)^T : per out-chunk m, accumulate over k
                for half in range(2):
                    ep = ps()
                    for j in range(3):
                        m = half * 3 + j
                        for k in range(KD):
                            wad2t = wad_p.tile([P, P], F32, tag="adw2t", bufs=2)
                            nc.sync.dma_start(
                                out=wad2t,
                                in_=adw2_d[k * P:(k + 1) * P, m * P:(m + 1) * P])
                            nc.tensor.matmul(ep[:, j * 512:j * 512 + 1], wad2t,
                                             a1T[:, k:k + 1],
                                             start=k == 0, stop=k == KD - 1)
                    nc.vector.tensor_copy(
                        out=embT[:, half * 3:half * 3 + 3],
                        in_=ep.rearrange("p (a b) -> p a b", b=512)[:, :, 0])

            # ---------------- helpers ----------------
            def prep_ln(t0, tn, pr_p, lnx8):
                """LN stats over features for tokens [t0, t0+tn);
                writes lnx8[:, :, t0:t0+tn]."""
                nh = tn // 2
                xTb = pr_p.tile([P, KD, T], BF16, tag="xtb", name="xTb")
                for c in range(KD):
                    if c % 2 == 0:
                        nc.scalar.copy(out=xTb[:, c, 0:tn],
                                       in_=xT[:, c, t0:t0 + tn])
                    else:
                        nc.vector.tensor_copy(out=xTb[:, c, 0:tn],
                                              in_=xT[:, c, t0:t0 + tn])
                sc1 = pr_p.tile([32, T], F32, tag="sc1", name="sc1")
                nc.vector.memset(sc1, 0.0)
                sc2 = pr_p.tile([32, T], F32, tag="sc2", name="sc2")
                mr_bc = pr_p.tile([P, 2, HB], F32, tag="mrbc", name="mr_bc")
                for half in range(2):
                    h0 = half * nh
                    # stats into sc1 row0: E[x] at [0:nh], E[x^2] at [HB:HB+nh]
                    sx_ps, sxx_ps = ps(), ps()
                    for c in range(KD):
                        sq = pr_p.tile([P, HB], BF16, tag="sq", bufs=2, name="sq")
                        nc.scalar.square(sq[:, 0:nh], xTb[:, c, h0:h0 + nh])
                        for g in range(nh // 512):
                            gs = slice(g * 512, (g + 1) * 512)
                            xs = slice(h0 + g * 512, h0 + (g + 1) * 512)
                            nc.tensor.matmul(sx_ps[0:1, gs], invd, xTb[:, c, xs],
                                             start=c == 0, stop=c == KD - 1)
                            nc.tensor.matmul(sxx_ps[0:1, gs], invd, sq[:, gs],
                                             start=c == 0, stop=c == KD - 1)
                    nc.scalar.copy(out=sc1[0:1, 0:nh], in_=sx_ps[0:1, 0:nh])
                    nc.scalar.copy(out=sc1[0:1, HB:HB + nh], in_=sxx_ps[0:1, 0:nh])
                    if nh == HB:
                        nc.vector.transpose(sc2[:, 0:2 * HB], sc1[:, 0:2 * HB])
                    else:
                        nc.vector.transpose(sc2[:, 0:nh], sc1[:, 0:nh])
                        nc.vector.transpose(sc2[:, HB:HB + nh],
                                            sc1[:, HB:HB + nh])
                    v = sc2.rearrange("p (b s) -> p b s", s=32)
                    nbh = nh // 32
                    B0 = HB // 32
                    m_ap = v[:, 0:nbh, 0]
                    r_ap = v[:, B0:B0 + nbh, 0]
                    t2 = v[:, 0:nbh, 2]
                    t3 = v[:, 0:nbh, 3]
                    # var = E[x^2] - m^2 ; newton rsqrt in place
                    nc.vector.tensor_tensor(out=t2, in0=m_ap, in1=m_ap,
                                            op=OP.mult)
                    nc.vector.tensor_tensor(out=r_ap, in0=r_ap, in1=t2,
                                            op=OP.subtract)
                    nc.vector.tensor_scalar(out=t2, in0=r_ap, scalar1=1e-5,
                                            scalar2=None, op0=OP.add)
                    yi = r_ap.bitcast(I32)
                    vi = t2.bitcast(I32)
                    nc.vector.tensor_scalar(out=yi, in0=vi, scalar1=1,
                                            scalar2=None,
                                            op0=OP.arith_shift_right)
                    nc.vector.tensor_scalar(out=yi, in0=yi, scalar1=-1,
                                            scalar2=0x5f3759df, op0=OP.mult,
                                            op1=OP.add)
                    nc.vector.tensor_scalar(out=t2, in0=t2, scalar1=0.5,
                                            scalar2=None, op0=OP.mult)
                    for _ in range(3):
                        nc.vector.tensor_tensor(out=t3, in0=r_ap, in1=r_ap,
                                                op=OP.mult)
                        nc.vector.tensor_tensor(out=t3, in0=t3, in1=t2,
                                                op=OP.mult)
                        nc.vector.tensor_scalar(out=t3, in0=t3, scalar1=-1.0,
                                                scalar2=1.5, op0=OP.mult,
                                                op1=OP.add)
                        nc.vector.tensor_tensor(out=r_ap, in0=r_ap, in1=t3,
                                                op=OP.mult)
                    if nh == HB:
                        nc.vector.transpose(sc1[:, 0:2 * HB], sc2[:, 0:2 * HB])
                    else:
                        nc.vector.transpose(sc1[:, 0:nh], sc2[:, 0:nh])
                        nc.vector.transpose(sc1[:, HB:HB + nh],
                                            sc2[:, HB:HB + nh])
                    src_mr = sc1[0:1, 0:2 * HB].rearrange(
                        "p (two h) -> p two h", two=2)[:, :, 0:nh]
                    nc.gpsimd.partition_broadcast(mr_bc[:, :, 0:nh], src_mr)
                    for g in range(nh // 512):
                        bs = slice(g * 512, (g + 1) * 512)
                        for c in range(KD):
                            xs = slice(t0 + h0 + g * 512,
                                       t0 + h0 + (g + 1) * 512)
                            tmp = pr_p.tile([P, HB], BF16, tag="sq", bufs=2,
                                            name="lntmp")
                            nc.vector.tensor_tensor(out=tmp[:, 0:512],
                                                    in0=xT[:, c, xs],
                                                    in1=mr_bc[:, 0, bs],
                                                    op=OP.subtract)
                            nc.vector.tensor_tensor(out=lnx8[:, c, xs],
                                                    in0=tmp[:, 0:512],
                                                    in1=mr_bc[:, 1, bs],
                                                    op=OP.mult)

            def fm_proj(w_sb, src8, foc, cols_src, out_ps, f8, nk=KD):
                """out_ps[:, 0:len] = (W^T src)[foc-chunk] ; stream cols_src."""
                n_g = (cols_src.stop - cols_src.start) // 512
                kc = kchunks(nk, f8)
                for ci, (c0, n) in enumerate(kc):
                    lhsT = s3(w_sb, c0, n, slice(foc * P, (foc + 1) * P))
                    for g in range(n_g):
                        rhs = s3(src8, c0, n,
                                 slice(cols_src.start + g * 512,
                                       cols_src.start + (g + 1) * 512))
                        cmm(out_ps[:, g * 512:(g + 1) * 512], lhsT, rhs,
                            start=ci == 0, stop=ci == len(kc) - 1, n=n)

            # ================= main =================
            for rep in range(repeat):
                # ---- ingest x0 (token-major DMA + PE transpose) ----
                with tc.tile_pool(name="ing", bufs=1) as ing_p:
                    for t in range(NT):
                        xin = ing_p.tile([P, D], F32, tag="xin", bufs=3, name="xin")
                        nc.sync.dma_start(out=xin, in_=x0_d[t * P:(t + 1) * P, :])
                        for half in range(2):
                            tp = ps()
                            for j in range(3):
                                c = half * 3 + j
                                tr128(tp[:, j * 512:j * 512 + P],
                                      xin[:, c * P:(c + 1) * P])
                            nc.vector.tensor_copy(
                                out=xT[:, half * 3:half * 3 + 3,
                                       t * P:(t + 1) * P],
                                in_=tp.rearrange("p (a b) -> p a b", b=512)[:, :, 0:P])

                phase_i = 0
                def action_rows(l):
                    # vrow/M/msum for layer l -> msumT (no xT dependency)
                    with tc.tile_pool(name="act_l", bufs=1) as al_p:
                        emb16 = al_p.tile([P, KD], BF16, name="emb16")
                        nc.vector.tensor_copy(out=emb16, in_=embT)
                        row_ps = ps()
                        for c in range(KD):
                            wt = al_p.tile([P, D], BF16, tag="wrow", bufs=2,
                                           name="wrow2")
                            nc.sync.dma_start(out=wt,
                                              in_=av_d[l, c * P:(c + 1) * P, :])
                            nc.tensor.matmul(row_ps[0:1, 0:512], emb16[:, c:c + 1],
                                             wt[:, 0:512],
                                             start=c == 0, stop=c == KD - 1)
                            nc.tensor.matmul(row_ps[0:1, 512:D], emb16[:, c:c + 1],
                                             wt[:, 512:D],
                                             start=c == 0, stop=c == KD - 1)
                        vrow = al_p.tile([1, D], F32, name="vrow")
                        nc.vector.tensor_copy(out=vrow, in_=row_ps[0:1, 0:D])
                        # v_bd [P, KD, H] bf16 block-diag
                        v_bd = al_p.tile([P, KD, H], BF16, name="v_bd")
                        nc.vector.memset(v_bd, 0.0)
                        for half in range(2):
                            tp = ps()
                            for j in range(3):
                                c = half * 3 + j
                                tr128(tp[:, j * 512:j * 512 + 1],
                                      vrow[0:1, c * P:(c + 1) * P])
                            for j in range(3):
                                c = half * 3 + j
                                nc.vector.tensor_copy(
                                    out=v_bd[0:64, c, 2 * c:2 * c + 1],
                                    in_=tp[0:64, j * 512:j * 512 + 1])
                                nc.vector.tensor_copy(
                                    out=v_bd[64:P, c, 2 * c + 1:2 * c + 2],
                                    in_=tp[64:P, j * 512:j * 512 + 1])
                        # M = v_bd^T @ a_cw  [12, D]
                        m_ps = ps()
                        for c in range(KD):
                            wt = al_p.tile([P, D], BF16, tag="wrow", bufs=2,
                                           name="wrow3")
                            nc.sync.dma_start(out=wt,
                                              in_=ac_d[l, c * P:(c + 1) * P, :])
                            nc.tensor.matmul(m_ps[0:12, 0:512], v_bd[:, c, :],
                                             wt[:, 0:512],
                                             start=c == 0, stop=c == KD - 1)
                            nc.tensor.matmul(m_ps[0:12, 512:D], v_bd[:, c, :],
                                             wt[:, 512:D],
                                             start=c == 0, stop=c == KD - 1)
                        nc.vector.tensor_copy(out=M_sb, in_=m_ps[0:12, 0:D])
                        # msumT[:, c] = sum_h M[h, c*128:(c+1)*128]
                        ms_ps = ps()
                        for c in range(KD):
                            nc.tensor.matmul(ms_ps[:, c:c + 1],
                                             M_sb[0:12, c * P:(c + 1) * P],
                                             onesb[0:12, :],
                                             start=c == 0, stop=c == KD - 1,
                                             skip_group_check=True)
                        nc.vector.tensor_copy(out=msumT, in_=ms_ps[:, 0:KD])

                action_rows(0)
                for l in range(n_layers):
                    # ======== action attention: x += colsum(M) ========
                    for c in range(KD):
                        nc.vector.tensor_scalar(out=xT[:, c, :], in0=xT[:, c, :],
                                                scalar1=msumT[:, c:c + 1],
                                                scalar2=None, op0=OP.add)
                    if dump == phase_i:
                        nc.sync.dma_start(out=dbg_d, in_=xT)
                    phase_i += 1

                    # ======== ln2 ========
                    with tc.tile_pool(name="prep", bufs=1) as pr_p:
                        prep_ln(0, T, pr_p, lnxa)
                    if dump == phase_i:
                        nc.sync.dma_start(out=dbg_d, in_=xT)
                    phase_i += 1

                    # ======== self-attn pass1: k/v/kvm ========
                    with tc.tile_pool(name="p1", bufs=1) as p1_p, \
                         tc.tile_pool(name="ps_kvm", bufs=1, space="PSUM") as kv_pp:
                        wk = p1_p.tile([P, KD, D], ADT, tag="wk", name="wk")
                        nc.sync.dma_start(
                            out=wk, in_=sk_d[l].rearrange("(c p) n -> p c n", p=P))
                        wv = p1_p.tile([P, KD, D], ADT, tag="wv", name="wv")
                        nc.sync.dma_start(
                            out=wv, in_=sv_d[l].rearrange("(c p) n -> p c n", p=P))
                        kvm_ps = kv_pp.tile([P, 1024], F32, name="kvm_ps")
                        # pre-set ones columns in the two vaug slots
                        vaug_slots = []
                        for i in range(2):
                            va = p1_p.tile([P, NPR, 130], ADT, tag="vaug", bufs=2,
                                           name="vaug")
                            nc.vector.memset(va[:, :, 64:65], 1.0)
                            nc.vector.memset(va[:, :, 129:130], 1.0)
                            vaug_slots.append(va)
                        kc = kchunks(KD, a8)
                        for t in range(NT):
                            tsl = slice(t * P, (t + 1) * P)
                            kv_ps = ps()
                            for ci, (c0, n) in enumerate(kc):
                                lhsT = s3(lnxa, c0, n, tsl)
                                first, last = ci == 0, ci == len(kc) - 1
                                # k cols 0:768, v cols 768:1536 (bank1 shared)
                                cmm(kv_ps[:, 0:512], lhsT,
                                    s3(wk, c0, n, slice(0, 512)),
                                    start=first, stop=last, n=n)
                                cmm(kv_ps[:, 512:768], lhsT,
                                    s3(wk, c0, n, slice(512, 768)),
                                    start=first, stop=False, n=n, skip=True)
                                cmm(kv_ps[:, 768:1024], lhsT,
                                    s3(wv, c0, n, slice(0, 256)),
                                    start=False, stop=last, n=n, skip=True)
                                cmm(kv_ps[:, 1024:1536], lhsT,
                                    s3(wv, c0, n, slice(256, 768)),
                                    start=first, stop=last, n=n)
                            # k epilogue: elu+1 -> k8
                            kmin = p1_p.tile([P, D], BF16, tag="kmin", bufs=2,
                                             name="kmin")
                            nc.scalar.activation(out=kmin, in_=kv_ps[:, 0:D],
                                                 func=AF.Relu, bias=0.0,
                                                 scale=-1.0)
                            kex = p1_p.tile([P, D], BF16, tag="kex", bufs=2,
                                            name="kex")
                            nc.scalar.activation(out=kex, in_=kmin, func=AF.Exp,
                                                 bias=0.0, scale=-1.0)
                            k8 = p1_p.tile([P, D], ADT, tag="k8", bufs=2, name="k8")
                            nc.vector.scalar_tensor_tensor(
                                out=k8, in0=kv_ps[:, 0:D], scalar=0.0, in1=kex,
                                op0=OP.max, op1=OP.add)
                            # v epilogue -> vaug (even/odd heads)
                            va = vaug_slots[t % 2]
                            vv = kv_ps[:, 768:1536].rearrange(
                                "p (pr two e) -> p pr two e", two=2, e=64)
                            nc.scalar.copy(out=va[:, :, 0:64], in_=vv[:, :, 0, :])
                            nc.scalar.copy(out=va[:, :, 65:129], in_=vv[:, :, 1, :])
                            # kvm accumulate
                            for pr in range(NPR):
                                off = kvoff(pr)
                                nc.tensor.matmul(
                                    kvm_ps[:, off:off + 130],
                                    k8[:, pr * P:(pr + 1) * P],
                                    va[:, pr, :],
                                    start=(t == 0 and pr % 3 == 0),
                                    stop=(t == NT - 1 and pr % 3 == 2),
                                    skip_group_check=True)
                        # kvm drains
                        nc.vector.tensor_scalar(out=kvm8[:, 0:390],
                                                in0=kvm_ps[:, 0:390], scalar1=SKVM,
                                                scalar2=None, op0=OP.mult)
                        nc.vector.tensor_scalar(out=kvm8[:, 512:902],
                                                in0=kvm_ps[:, 512:902], scalar1=SKVM,
                                                scalar2=None, op0=OP.mult)
                        nc.vector.memset(kbd, 0.0)
                        # ksum columns -> block-diag kbd (scaled)
                        for pr in range(NPR):
                            off = kvoff(pr)
                            nc.vector.tensor_scalar(
                                out=kbd[0:64, 2 * pr:2 * pr + 1],
                                in0=kvm_ps[0:64, off + 64:off + 65],
                                scalar1=SKBD, scalar2=None, op0=OP.mult)
                            nc.vector.tensor_scalar(
                                out=kbd[64:P, 2 * pr + 1:2 * pr + 2],
                                in0=kvm_ps[64:P, off + 129:off + 130],
                                scalar1=SKBD, scalar2=None, op0=OP.mult)

                    # ======== self-attn pass2: q/den/y/c ========
                    with tc.tile_pool(name="p2", bufs=1) as p2_p:
                        wq = p2_p.tile([P, KD, D], ADT, tag="wq", name="wq")
                        nc.sync.dma_start(
                            out=wq, in_=sq_d[l].rearrange("(c p) n -> p c n", p=P))
                        wc = p2_p.tile([P, KD, D], ADT, tag="wc", name="wc")
                        nc.sync.dma_start(
                            out=wc, in_=sc_d[l].rearrange("(c p) n -> p c n", p=P))
                        for half in range(NSP):
                            hsl = slice(half * SPAN, (half + 1) * SPAN)
                            q8 = p2_p.tile([P, KD, SPAN], ADT, tag="q8", bufs=2,
                                           name="q8")
                            for foc in range(KD):
                                q_ps = ps()
                                fm_proj(wq, lnxa, foc, hsl, q_ps, a8)
                                qmin = p2_p.tile([P, SPAN], BF16, tag="qmin",
                                                 bufs=2, name="qmin")
                                nc.scalar.activation(out=qmin,
                                                     in_=q_ps[:, 0:SPAN],
                                                     func=AF.Relu, bias=0.0,
                                                     scale=-1.0)
                                qex = p2_p.tile([P, SPAN], BF16, tag="qex", bufs=2,
                                                name="qex")
                                nc.scalar.activation(out=qex, in_=qmin, func=AF.Exp,
                                                     bias=0.0, scale=-1.0)
                                nc.vector.scalar_tensor_tensor(
                                    out=q8[:, foc, :], in0=q_ps[:, 0:SPAN],
                                    scalar=0.0, in1=qex, op0=OP.max, op1=OP.add)
                            # den token-major for all pairs -> one recip
                            NTT = SPAN // P
                            dt_ps = ps()
                            nmm = NPR * NTT
                            i = 0
                            for pr in range(NPR):
                                for tt in range(NTT):
                                    co = pr * 2 * NTT + 2 * tt
                                    nc.tensor.matmul(
                                        dt_ps[:, co:co + 2],
                                        q8[:, pr, tt * P:(tt + 1) * P],
                                        kbd[:, 2 * pr:2 * pr + 2],
                                        start=(i == 0), stop=(i == nmm - 1),
                                        skip_group_check=True)
                                    i += 1
                            ztr = p2_p.tile([P, 2 * NTT * NPR], F32, tag="ztr",
                                            name="ztr")
                            nc.vector.tensor_scalar(out=ztr,
                                                    in0=dt_ps[:, 0:2 * nmm],
                                                    scalar1=1.0 / SKBD,
                                                    scalar2=1e-6,
                                                    op0=OP.mult, op1=OP.add)
                            nc.vector.reciprocal(out=ztr, in_=ztr)
                            y8 = p2_p.tile([P, KD, SPAN], ADT, tag="y8", bufs=2,
                                           name="y8")
                            for pr in range(NPR):
                                zt_ps = ps()
                                for tt in range(NTT):
                                    co = pr * 2 * NTT + 2 * tt
                                    tr128(zt_ps[0:2, tt * P:(tt + 1) * P],
                                          ztr[:, co:co + 2])
                                zrow16 = p2_p.tile([2, SPAN], BF16, tag="zrow16",
                                                   bufs=2, name="zrow16")
                                nc.scalar.copy(out=zrow16,
                                               in_=zt_ps[0:2, 0:SPAN])
                                zb_ps = ps()
                                for g in range(NG):
                                    gs = slice(g * 512, (g + 1) * 512)
                                    nc.tensor.matmul(zb_ps[:, gs], sel16,
                                                     zrow16[0:2, gs],
                                                     start=True, stop=True)
                                zbc = p2_p.tile([P, SPAN], BF16, tag="zbc", bufs=2,
                                                name="zbc")
                                nc.scalar.copy(out=zbc, in_=zb_ps[:, 0:SPAN])
                                off = kvoff(pr)
                                y_ps = ps()
                                for g in range(NG):
                                    gs = slice(g * 512, (g + 1) * 512)
                                    nc.tensor.matmul(y_ps[0:64, gs],
                                                     kvm8[0:64, off:off + 64],
                                                     q8[0:64, pr, gs],
                                                     start=True, stop=True)
                                    nc.tensor.matmul(y_ps[64:P, gs],
                                                     kvm8[64:P, off + 65:off + 129],
                                                     q8[64:P, pr, gs],
                                                     start=True, stop=True)
                                nc.vector.scalar_tensor_tensor(
                                    out=y8[:, pr, :], in0=y_ps[:, 0:SPAN],
                                    scalar=1.0 / SKVM,
                                    in1=zbc, op0=OP.mult, op1=OP.mult)
                            # c-proj + residual
                            for foc in range(KD):
                                o_ps = ps()
                                fm_proj(wc, y8, foc, slice(0, SPAN), o_ps, a8)
                                nc.vector.tensor_tensor(
                                    out=xT[:, foc, hsl], in0=xT[:, foc, hsl],
                                    in1=o_ps[:, 0:SPAN], op=OP.add)
                    if dump == phase_i:
                        nc.sync.dma_start(out=dbg_d, in_=xT)
                    phase_i += 1

                    if l + 1 < n_layers:
                        action_rows(l + 1)   # fills the ln3 prep gap
                    # ======== ln3 ========
                    with tc.tile_pool(name="prep", bufs=1) as pr_p:
                        prep_ln(0, T, pr_p, lnxm)
                    phase_i += 1

                    # ======== MLP ========
                    with tc.tile_pool(name="mlp", bufs=1) as ml_p:
                        fkc = kchunks(FC, m8)
                        for half in range(NSPM):
                            hsl = slice(half * SPANM, (half + 1) * SPANM)
                            h8 = ml_p.tile([P, FC, SPANM], MDT, tag="h8", name="h8")
                            for fo in range(FC):
                                w1t = ml_p.tile([P, KD, P], MDT, tag="w1c", bufs=3,
                                                name="w1c")
                                nc.sync.dma_start(
                                    out=w1t,
                                    in_=w1_d[l, :, fo * P:(fo + 1) * P]
                                    .rearrange("(c p) n -> p c n", p=P))
                                h_ps = ps()
                                fm_proj(w1t, lnxm, 0, hsl, h_ps, m8)
                                nc.scalar.activation(out=h8[:, fo, :],
                                                     in_=h_ps[:, 0:SPANM],
                                                     func=AF.Gelu_apprx_tanh,
                                                     bias=0.0, scale=1.0)
                            for fod in range(KD):
                                w2t = ml_p.tile([P, FC, P], MDT, tag="w2c", bufs=2,
                                                name="w2c")
                                nc.sync.dma_start(
                                    out=w2t,
                                    in_=w2_d[l, :, fod * P:(fod + 1) * P]
                                    .rearrange("(c p) n -> p c n", p=P))
                                o_ps = ps()
                                for ci, (c0, n) in enumerate(fkc):
                                    lhsT = s3(w2t, c0, n, slice(0, P))
                                    for g in range(NGM):
                                        gs = slice(g * 512, (g + 1) * 512)
                                        cmm(o_ps[:, gs], lhsT,
                                            s3(h8, c0, n, gs),
                                            start=ci == 0, stop=ci == len(fkc) - 1,
                                            n=n)
                                nc.vector.tensor_tensor(
                                    out=xT[:, fod, hsl], in0=xT[:, fod, hsl],
                                    in1=o_ps[:, 0:SPANM], op=OP.add)
                    if dump == phase_i:
                        nc.sync.dma_start(out=dbg_d, in_=xT)
                    phase_i += 1

                # ======== final LN + heads ========
                with tc.tile_pool(name="prep", bufs=1) as pr_p:
                    prep_ln(CL, TT, pr_p, lnx16)
                with tc.tile_pool(name="fh", bufs=1) as fh_p:
                    wmu = fh_p.tile([P, KD, D], BF16, tag="wmu", name="wmu")
                    nc.sync.dma_start(
                        out=wmu, in_=wmu_d.rearrange("(c p) n -> p c n", p=P))
                    wlv = fh_p.tile([P, KD, D], BF16, tag="wlv", name="wlv")
                    nc.sync.dma_start(
                        out=wlv, in_=wlv_d.rearrange("(c p) n -> p c n", p=P))
                    kc = kchunks(KD, False)
                    for t in range(16, NT):
                        tsl = slice(CL + (t - 16) * P, CL + (t - 15) * P)
                        mu_ps = ps()
                        for ci, (c0, n) in enumerate(kc):
                            lhsT = s3(lnx16, c0, n, tsl)
                            first, last = ci == 0, ci == len(kc) - 1
                            cmm(mu_ps[:, 0:512], lhsT, s3(wmu, c0, n, slice(0, 512)),
                                start=first, stop=last, n=n)
                            cmm(mu_ps[:, 512:768], lhsT,
                                s3(wmu, c0, n, slice(512, 768)),
                                start=first, stop=False, n=n, skip=True)
                            cmm(mu_ps[:, 768:1024], lhsT,
                                s3(wlv, c0, n, slice(0, 256)),
                                start=False, stop=last, n=n, skip=True)
                            cmm(mu_ps[:, 1024:1536], lhsT,
                                s3(wlv, c0, n, slice(256, 768)),
                                start=first, stop=last, n=n)
                        r0 = (t - 16) * P
                        mu_sb = fh_p.tile([P, D], F32, tag="mu_sb", bufs=2,
                                          name="mu_sb")
                        nc.scalar.copy(out=mu_sb, in_=mu_ps[:, 0:D])
                        nc.sync.dma_start(out=mu_d[r0:r0 + P, :], in_=mu_sb)
                        lv_sb = fh_p.tile([P, D], F32, tag="lv_sb", bufs=2,
                                          name="lv_sb")
                        nc.vector.tensor_scalar(out=lv_sb, in0=mu_ps[:, D:2 * D],
                                                scalar1=-10.0, scalar2=2.0,
                                                op0=OP.max, op1=OP.min)
                        nc.sync.dma_start(out=lv_d[r0:r0 + P, :], in_=lv_sb)

    nc.finalize()
    return nc


_NC_CACHE = {}


def _get_nc(mode, repeat):
    key = (mode, repeat)
    if key not in _NC_CACHE:
        _NC_CACHE[key] = build_nc(mode, repeat)
    return _NC_CACHE[key]


def make_in_maps(inputs, mode=MODE):
    import ml_dtypes
    a8 = mode == 'full8'
    m8 = mode in ('mlp8', 'full8')
    adt = ml_dtypes.float8_e4m3 if a8 else ml_dtypes.bfloat16
    mdt = ml_dtypes.float8_e4m3 if m8 else ml_dtypes.bfloat16
    b16 = ml_dtypes.bfloat16
    ctx = np.asarray(inputs['context_latents'], np.float32)
    acts = np.asarray(inputs['action_latents'], np.float32)
    idx = np.asarray(inputs['target_indices'])
    mq = np.asarray(inputs['mq'], np.float32)

    adw1 = np.zeros((A_PAD, D), np.float32)
    adw1[:320, :] = np.asarray(inputs['ad_w1'], np.float32)

    def cv(name, dt):
        return np.ascontiguousarray(np.asarray(inputs[name]).astype(dt))

    shared = {
        'adw1': adw1,
        'adw2': np.asarray(inputs['ad_w2'], np.float32),
        'av': cv('a_vw', b16), 'ac': cv('a_cw', b16),
        'sq': cv('s_qw', adt), 'sk': cv('s_kw', adt),
        'sv': cv('s_vw', adt), 'sc': cv('s_cw', adt),
        'w1': cv('mlp_w1', mdt), 'w2': cv('mlp_w2', mdt),
        'wmu': cv('mu_w', b16), 'wlv': cv('lv_w', b16),
    }
    in_maps = []
    for b in range(8):
        x0 = np.concatenate([ctx[b], mq[idx[b]]], axis=0)
        a = np.zeros((A_PAD, 1), np.float32)
        a[:320, 0] = acts[b]
        in_maps.append({'x0': np.ascontiguousarray(x0), 'act': a, **shared})
    return in_maps


def kernel(**inputs):
    nc = _get_nc(MODE, REPEAT)
    in_maps = make_in_maps(inputs, MODE)
    r = run_bass_kernel_spmd(nc, in_maps, list(range(8)))
    mu = np.stack([r.results[b]['mu'] for b in range(8)])
    lv = np.stack([r.results[b]['lv'] for b in range(8)])
    return mu, lv
